# revision 17
# baseline (speedup 1.0000x reference)
"""GIN message-passing (3-layer) + per-graph scatter softmax on 8 Trainium2 cores.

Strategy (graph/data parallel per the sharding hint, adapted to the fact that
edges here are uniformly random across the whole node set):
  - dst-shard nodes: core c owns nodes [c*N/8, (c+1)*N/8).
  - host: append self-loops (GIN eps=0 means z = h + sum_in == aggregation with
    a self-edge), sort edges by dst, bucket them into 128-node dst blocks,
    pad each block's edge list to K tiles of 128 edges (K = global max so the
    SPMD instruction stream is identical on all cores).
  - device, per layer: for each block, one big indirect-DMA gather pulls the
    src-node feature rows ([128 edges x 128 feat] per tile); a one-hot
    "selection" matrix S (built with a single broadcasted is_equal per block)
    maps edges to their dst slot; PE matmuls accumulate agg^T = gathered^T
    one-hot in PSUM.  The GIN MLP runs on the transposed [feat, node] block,
    BatchNorm batch stats are reduced on-chip and AllReduce'd across cores
    (tiny), applied fused with ReLU on the scalar engine, and the new node
    features are transposed back and AllGather'd so every core has the full
    feature table for the next layer's gathers.
  - final layer produces [1, n] logits; per-graph softmax is done with
    one-hot graph-membership matmuls + one tiny AllReduce.
"""

import sys

import numpy as np

for _p in ("/opt/trn_rl_repo",):
    if _p not in sys.path:
        sys.path.insert(0, _p)

from contextlib import ExitStack

import concourse.bass as bass
import concourse.tile as tile
from concourse import bacc, bass_utils, mybir
from concourse.masks import make_identity

P = 128
NCORES = 8
F32 = mybir.dt.float32
I32 = mybir.dt.int32
RG = [list(range(NCORES))]
BN_EPS = 1e-5


def _cfg(n_nodes, n_graphs, nsub=None):
    npc = n_nodes // NCORES
    assert npc * NCORES == n_nodes
    nblk = -(-npc // P)
    nlast = npc - (nblk - 1) * P
    if nsub is None:
        nsub = max(1, -(-n_nodes // 32000))
    assert n_nodes % nsub == 0 and n_nodes // nsub <= 32767
    return dict(N=n_nodes, G=n_graphs, NPC=npc, NBLK=nblk, NLAST=nlast, NSUB=nsub)


def _preprocess(edge_index, batch, cfg):
    """Build per-core gather indices / relative-dst tables / graph-id tables.

    dma_gather takes int16 indices, so the node table is viewed as NSUB
    row-slices ("subtables") and each 128-node dst block issues one gather per
    subtable.  Indices for one call are laid out in the SWDGE wrap: index i at
    [partition i%16, col i//16], replicated across the 8 Q7 16-partition
    groups.  Pad slots use index 0 (always valid; the selection matrix zeroes
    their contribution via dst_rel = -1).
    """
    N, NPC, NBLK, NSUB = cfg["N"], cfg["NPC"], cfg["NBLK"], cfg["NSUB"]
    SUBN = N // NSUB
    src = np.asarray(edge_index[0], dtype=np.int64)
    dst = np.asarray(edge_index[1], dtype=np.int64)
    # self loops implement the "+ h_i" of GINConv(eps=0)
    loops = np.arange(N, dtype=np.int64)
    src = np.concatenate([src, loops])
    dst = np.concatenate([dst, loops])

    core = dst // NPC
    loc = dst - core * NPC
    blk = loc // P
    rel = loc - blk * P
    sub = src // SUBN
    key = (core * NBLK + blk) * NSUB + sub
    order = np.argsort(key, kind="stable")
    src, rel, key, sub = src[order], rel[order], key[order], sub[order]

    counts = np.bincount(key, minlength=NCORES * NBLK * NSUB)
    K = int(-(-counts.max() // P))  # tiles per (block, subtable) call
    slot = K * P

    idx_pad = np.zeros((NCORES * NBLK * NSUB, slot), np.int16)
    rel_pad = np.full((NCORES * NBLK * NSUB, slot), -1.0, np.float32)
    starts = np.zeros(NCORES * NBLK * NSUB + 1, np.int64)
    starts[1:] = np.cumsum(counts)
    pos = np.arange(len(src)) - starts[key]
    idx_pad[key, pos] = (src - sub * SUBN).astype(np.int16)
    rel_pad[key, pos] = rel.astype(np.float32)

    # dst_rel: edge slot i of call (b,s) -> tile t=i//128, partition p=i%128,
    # at column b*(NSUB*K) + s*K + t
    dst_rel = (
        rel_pad.reshape(NCORES, NBLK * NSUB, K, P)
        .transpose(0, 3, 1, 2)
        .reshape(NCORES, P, NBLK * NSUB * K)
    )
    # int16 indices in SWDGE wrap per call: [16, slot//16] replicated 8x
    wrap = idx_pad.reshape(NCORES, NBLK * NSUB, slot // 16, 16).transpose(0, 3, 1, 2)
    wrap = wrap.reshape(NCORES, 1, 16, NBLK * NSUB * (slot // 16))
    idx16 = np.broadcast_to(wrap, (NCORES, 8, 16, NBLK * NSUB * (slot // 16))).reshape(
        NCORES, P, NBLK * NSUB * (slot // 16)
    )

    # graph id of node b*128+p of each core, -1 for pad nodes
    bfull = np.asarray(batch, dtype=np.int64)
    gid = np.full((NCORES, NBLK * P), -1.0, np.float32)
    for c in range(NCORES):
        gid[c, :NPC] = bfull[c * NPC : (c + 1) * NPC].astype(np.float32)
    gid = gid.reshape(NCORES, NBLK, P).transpose(0, 2, 1)  # [NCORES, P, NBLK]

    return np.ascontiguousarray(idx16), np.ascontiguousarray(dst_rel), np.ascontiguousarray(gid), K


def _build_nc(cfg, K):
    N, G, NPC, NBLK, NLAST = cfg["N"], cfg["G"], cfg["NPC"], cfg["NBLK"], cfg["NLAST"]  # noqa: F841
    NSUB = cfg["NSUB"]
    SUBN = N // NSUB
    KT = NSUB * K  # edge tiles per 128-node block
    IW = K * 8  # int16 idx columns per gather call
    nc = bacc.Bacc("TRN2", target_bir_lowering=False, debug=False, num_devices=NCORES)

    xd = nc.dram_tensor("x", [N, P], F32, kind="ExternalInput")
    srcd = nc.dram_tensor("src_idx", [P, NBLK * NSUB * IW], mybir.dt.int16, kind="ExternalInput")
    dreld = nc.dram_tensor("dst_rel", [P, NBLK * KT], F32, kind="ExternalInput")
    gidd = nc.dram_tensor("gid", [P, NBLK], F32, kind="ExternalInput")
    iotad = nc.dram_tensor("iota", [P, P], F32, kind="ExternalInput")
    w1d = [nc.dram_tensor(f"w1_{l}", [P, P], F32, kind="ExternalInput") for l in range(3)]
    w2d = [
        nc.dram_tensor(f"w2_{l}", [P, P if l < 2 else 1], F32, kind="ExternalInput")
        for l in range(3)
    ]
    b1d = [nc.dram_tensor(f"b1_{l}", [P, 1], F32, kind="ExternalInput") for l in range(3)]
    dout = [P, P, 1]
    b2d = [nc.dram_tensor(f"b2_{l}", [dout[l], 1], F32, kind="ExternalInput") for l in range(3)]
    gmd = [nc.dram_tensor(f"gm_{l}", [dout[l], 1], F32, kind="ExternalInput") for l in range(3)]
    btd = [nc.dram_tensor(f"bt_{l}", [dout[l], 1], F32, kind="ExternalInput") for l in range(3)]
    outd = nc.dram_tensor("out", [NBLK, P], F32, kind="ExternalOutput")

    with tile.TileContext(nc) as tc, ExitStack() as ctx:
        consts = ctx.enter_context(tc.tile_pool(name="consts", bufs=1))
        dram = ctx.enter_context(tc.tile_pool(name="dram", bufs=1, space="DRAM"))
        work = ctx.enter_context(tc.tile_pool(name="work", bufs=3))
        spool = ctx.enter_context(tc.tile_pool(name="spool", bufs=3))
        small = ctx.enter_context(tc.tile_pool(name="small", bufs=2))
        ps_agg = ctx.enter_context(tc.tile_pool(name="ps_agg", bufs=2, space="PSUM"))
        ps_mlp = ctx.enter_context(tc.tile_pool(name="ps_mlp", bufs=3, space="PSUM"))
        ps_aux = ctx.enter_context(tc.tile_pool(name="ps_aux", bufs=2, space="PSUM"))
        ps_acc = ctx.enter_context(tc.tile_pool(name="ps_acc", bufs=1, space="PSUM"))

        # ---- persistent SBUF state -------------------------------------
        src_sb = consts.tile([P, NBLK * NSUB * IW], mybir.dt.int16)
        nc.sync.dma_start(src_sb[:], srcd.ap())
        drel_sb = consts.tile([P, NBLK * KT], F32)
        nc.sync.dma_start(drel_sb[:], dreld.ap())
        gid_sb = consts.tile([P, NBLK], F32)
        nc.sync.dma_start(gid_sb[:], gidd.ap())
        iota_sb = consts.tile([P, P], F32)
        nc.sync.dma_start(iota_sb[:], iotad.ap())
        ident_sb = consts.tile([P, P], F32)
        make_identity(nc, ident_sb[:])
        ones_row = consts.tile([1, P], F32)
        nc.any.memset(ones_row[:], 1.0)
        epsv = consts.tile([P, 1], F32)
        nc.any.memset(epsv[:], BN_EPS)

        w1_sb, w2_sb, b1_sb, b2_sb, gm_sb, bt_sb = [], [], [], [], [], []
        for l in range(3):
            w1 = consts.tile([P, P], F32, name=f"w1sb{l}")
            nc.sync.dma_start(w1[:], w1d[l].ap())
            w1_sb.append(w1)
            w2 = consts.tile([P, dout[l] if l == 2 else P], F32, name=f"w2sb{l}")
            nc.sync.dma_start(w2[:], w2d[l].ap())
            w2_sb.append(w2)
            for lst, d in ((b1_sb, b1d), (b2_sb, b2d), (gm_sb, gmd), (bt_sb, btd)):
                t = consts.tile(list(d[l].shape), F32, name=f"p{id(d)}_{l}")
                nc.sync.dma_start(t[:], d[l].ap())
                lst.append(t)

        h_store = consts.tile([P, NBLK * P], F32)
        z3n = consts.tile([P, NBLK], F32)
        e_n = consts.tile([P, NBLK], F32)
        outn = consts.tile([P, NBLK], F32)

        ag_in = dram.tile([NPC, P], F32)

        def layer(l, h_table_ap):
            do = dout[l]
            # ---- phase A: aggregate + MLP per 128-node block ----------
            for b in range(NBLK):
                gb = work.tile([P, KT, P], F32, tag="gbuf", name=f"gb_{l}_{b}")
                for s in range(NSUB):
                    nc.gpsimd.dma_gather(
                        gb[:, s * K : (s + 1) * K, :],
                        h_table_ap[s * SUBN : (s + 1) * SUBN, :],
                        src_sb[:, (b * NSUB + s) * IW : (b * NSUB + s + 1) * IW],
                        K * P,
                        K * P,
                        P,
                    )
                smat = work.tile([P, KT, P], F32, tag="smat", bufs=2, name=f"sm_{l}_{b}")
                nc.vector.tensor_tensor(
                    out=smat[:],
                    in0=drel_sb[:, b * KT : (b + 1) * KT].to_broadcast([P, KT, P]),
                    in1=iota_sb[:][:, None, :].to_broadcast([P, KT, P]),
                    op=mybir.AluOpType.is_equal,
                )
                agg = ps_agg.tile([P, P], F32, tag="agg", name=f"agg_{l}_{b}")
                for t in range(KT):
                    nc.tensor.matmul(
                        agg[:], gb[:, t, :], smat[:, t, :], start=(t == 0), stop=(t == KT - 1)
                    )
                zsb = spool.tile([P, P], F32, tag="zsb", name=f"z_{l}_{b}")
                nc.vector.tensor_copy(zsb[:], agg[:])
                m1 = ps_mlp.tile([P, P], F32, tag="mlp", name=f"m1_{l}_{b}")
                nc.tensor.matmul(m1[:], w1_sb[l][:], zsb[:], start=True, stop=True)
                r1 = spool.tile([P, P], F32, tag="r1", name=f"r1_{l}_{b}")
                nc.scalar.activation(
                    r1[:], m1[:], mybir.ActivationFunctionType.Relu, bias=b1_sb[l][:]
                )
                m2 = ps_mlp.tile([do, P], F32, tag="mlp", name=f"m2_{l}_{b}")
                nc.tensor.matmul(m2[:], w2_sb[l][:, 0:do], r1[:], start=True, stop=True)
                nc.scalar.activation(
                    h_store[0:do, b * P : (b + 1) * P],
                    m2[:],
                    mybir.ActivationFunctionType.Identity,
                    bias=b2_sb[l][:],
                )
            # ---- phase B: batchnorm stats over this core's nodes ------
            ssum = small.tile([do, 1], F32, name=f"ssum{l}")
            nc.vector.tensor_reduce(
                out=ssum[:], in_=h_store[0:do, 0:NPC], axis=mybir.AxisListType.X,
                op=mybir.AluOpType.add,
            )
            nchunk = -(-NPC // 2048)
            ssq_parts = small.tile([do, nchunk], F32, name=f"ssqp{l}")
            sq_scr = work.tile([P, 2048], F32, tag="sqscr", bufs=1, name=f"sqs{l}")
            for i in range(nchunk):
                st = i * 2048
                ln = min(2048, NPC - st)
                nc.scalar.activation(
                    sq_scr[0:do, 0:ln],
                    h_store[0:do, st : st + ln],
                    mybir.ActivationFunctionType.Square,
                    accum_out=ssq_parts[:, i : i + 1],
                )
            ssq = small.tile([do, 1], F32, name=f"ssq{l}")
            nc.vector.tensor_reduce(
                out=ssq[:], in_=ssq_parts[:], axis=mybir.AxisListType.X,
                op=mybir.AluOpType.add,
            )
            stat2 = small.tile([do, 2], F32, name=f"st2{l}")
            nc.vector.tensor_copy(stat2[:, 0:1], ssum[:])
            nc.vector.tensor_copy(stat2[:, 1:2], ssq[:])
            stats_in = dram.tile([do, 2], F32, name=f"sti{l}")
            stats_out = dram.tile([do, 2], F32, addr_space="Shared", name=f"sto{l}")
            nc.sync.dma_start(stats_in[:], stat2[:])
            nc.gpsimd.collective_compute(
                "AllReduce", mybir.AluOpType.add, replica_groups=RG,
                ins=[stats_in[:]], outs=[stats_out[:]],
            )
            statr = small.tile([do, 2], F32, name=f"str{l}")
            nc.sync.dma_start(statr[:], stats_out[:])
            # ---- phase C: scale/shift ---------------------------------
            mu = small.tile([do, 1], F32, name=f"mu{l}")
            nc.scalar.mul(mu[:], statr[:, 0:1], 1.0 / N)
            ex2 = small.tile([do, 1], F32, name=f"ex2{l}")
            nc.scalar.mul(ex2[:], statr[:, 1:2], 1.0 / N)
            musq = small.tile([do, 1], F32, name=f"musq{l}")
            nc.scalar.square(musq[:], mu[:])
            var = small.tile([do, 1], F32, name=f"var{l}")
            nc.vector.tensor_tensor(
                out=var[:], in0=ex2[:], in1=musq[:], op=mybir.AluOpType.subtract
            )
            std = small.tile([do, 1], F32, name=f"std{l}")
            nc.scalar.activation(
                std[:], var[:], mybir.ActivationFunctionType.Sqrt, bias=epsv[0:do, :]
            )
            rstd = small.tile([do, 1], F32, name=f"rstd{l}")
            nc.vector.reciprocal(rstd[:], std[:])
            scal = small.tile([do, 1], F32, name=f"scal{l}")
            nc.vector.tensor_tensor(
                out=scal[:], in0=rstd[:], in1=gm_sb[l][:], op=mybir.AluOpType.mult
            )
            mus = small.tile([do, 1], F32, name=f"mus{l}")
            nc.vector.tensor_tensor(
                out=mus[:], in0=mu[:], in1=scal[:], op=mybir.AluOpType.mult
            )
            shift = small.tile([do, 1], F32, name=f"shift{l}")
            nc.vector.tensor_tensor(
                out=shift[:], in0=bt_sb[l][:], in1=mus[:], op=mybir.AluOpType.subtract
            )
            return scal, shift

        # ================= layers 0 and 1 ==========================
        h_tables = [xd.ap()]
        for l in range(2):
            scal, shift = layer(l, h_tables[l])
            # ---- phase D: BN+ReLU, transpose back, write shard, AllGather
            for b in range(NBLK):
                hb = spool.tile([P, P], F32, tag="hb", name=f"hb_{l}_{b}")
                nc.scalar.activation(
                    hb[:],
                    h_store[:, b * P : (b + 1) * P],
                    mybir.ActivationFunctionType.Relu,
                    bias=shift[:],
                    scale=scal[:],
                )
                tp = ps_aux.tile([P, P], F32, tag="aux", name=f"tp_{l}_{b}")
                nc.tensor.transpose(tp[:], hb[:], ident_sb[:])
                tpsb = spool.tile([P, P], F32, tag="tpsb", name=f"tb_{l}_{b}")
                nc.vector.tensor_copy(tpsb[:], tp[:])
                rows = P if b < NBLK - 1 else NLAST
                nc.sync.dma_start(ag_in[b * P : b * P + rows, :], tpsb[0:rows, :])
            h_next = dram.tile([N, P], F32, addr_space="Shared", name=f"hfull{l}")
            nc.gpsimd.collective_compute(
                "AllGather", mybir.AluOpType.bypass, replica_groups=RG,
                ins=[ag_in[:]], outs=[h_next[:]],
            )
            h_tables.append(h_next)

        # ================= layer 2 (dout=1) ========================
        scal3, shift3 = layer(2, h_tables[2])
        # fold the final /5 into the affine transform
        scal3f = small.tile([1, 1], F32, name="scal3f")
        nc.scalar.mul(scal3f[:], scal3[:], 0.2)
        shift3f = small.tile([1, 1], F32, name="shift3f")
        nc.scalar.mul(shift3f[:], shift3[:], 0.2)
        # broadcast the two scalars across partitions via 1-row matmuls
        bc1 = ps_aux.tile([P, 1], F32, tag="aux", name="bc1")
        nc.tensor.matmul(bc1[:], ones_row[:], scal3f[:], start=True, stop=True)
        scal3bc = small.tile([P, 1], F32, name="scal3bc")
        nc.vector.tensor_copy(scal3bc[:], bc1[:])
        bc2 = ps_aux.tile([P, 1], F32, tag="aux", name="bc2")
        nc.tensor.matmul(bc2[:], ones_row[:], shift3f[:], start=True, stop=True)
        shift3bc = small.tile([P, 1], F32, name="shift3bc")
        nc.vector.tensor_copy(shift3bc[:], bc2[:])
        # transpose the [1, n] logits into node-on-partition layout
        for b in range(NBLK):
            tp1 = ps_aux.tile([P, 1], F32, tag="aux", name=f"tp1_{b}")
            nc.tensor.transpose(
                tp1[:], h_store[0:1, b * P : (b + 1) * P], ident_sb[0:1, 0:1]
            )
            nc.vector.tensor_copy(z3n[:, b : b + 1], tp1[:])
        # e = exp(t) where t = scal3f*z3 + shift3f  (BN + /5 fused; max-shift
        # omitted: |t| is O(1) so exp is safe in f32)
        nc.scalar.activation(
            e_n[:], z3n[:], mybir.ActivationFunctionType.Exp,
            bias=shift3bc[:], scale=scal3bc[:],
        )
        # per-graph sums: accumulate G_b^T e_b over blocks in PSUM
        gsum = ps_acc.tile([G, 1], F32, name="gsum")
        for b in range(NBLK):
            gb1 = spool.tile([P, G], F32, tag="ghot", name=f"gh_{b}")
            nc.vector.tensor_tensor(
                out=gb1[:],
                in0=gid_sb[:, b : b + 1].to_broadcast([P, G]),
                in1=iota_sb[:, 0:G],
                op=mybir.AluOpType.is_equal,
            )
            nc.tensor.matmul(
                gsum[:], gb1[:], e_n[:, b : b + 1], start=(b == 0), stop=(b == NBLK - 1)
            )
        gsum_sb = small.tile([G, 1], F32, name="gsum_sb")
        nc.vector.tensor_copy(gsum_sb[:], gsum[:])
        gs_in = dram.tile([G, 1], F32, name="gs_in")
        gs_out = dram.tile([G, 1], F32, addr_space="Shared", name="gs_out")
        nc.sync.dma_start(gs_in[:], gsum_sb[:])
        nc.gpsimd.collective_compute(
            "AllReduce", mybir.AluOpType.add, replica_groups=RG,
            ins=[gs_in[:]], outs=[gs_out[:]],
        )
        s_all = small.tile([G, 1], F32, name="s_all")
        nc.sync.dma_start(s_all[:], gs_out[:])
        rs = small.tile([G, 1], F32, name="rs")
        nc.vector.reciprocal(rs[:], s_all[:])
        # out = e / s[graph]: denominator per node via G_b^T matmul
        for b in range(NBLK):
            gb2 = spool.tile([P, G], F32, tag="ghot", name=f"gh2_{b}")
            nc.vector.tensor_tensor(
                out=gb2[:],
                in0=gid_sb[:, b : b + 1].to_broadcast([P, G]),
                in1=iota_sb[:, 0:G],
                op=mybir.AluOpType.is_equal,
            )
            tpg = ps_aux.tile([G, P], F32, tag="aux", name=f"tpg_{b}")
            nc.tensor.transpose(tpg[:], gb2[:], ident_sb[:])
            gbt = spool.tile([G, P], F32, tag="gbt", name=f"gbt_{b}")
            nc.vector.tensor_copy(gbt[:], tpg[:])
            den = ps_mlp.tile([P, 1], F32, tag="mlp", name=f"den_{b}")
            nc.tensor.matmul(den[:], gbt[:], rs[:], start=True, stop=True)
            nc.vector.tensor_tensor(
                out=outn[:, b : b + 1], in0=e_n[:, b : b + 1], in1=den[:],
                op=mybir.AluOpType.mult,
            )
        # transpose [P, NBLK] -> [NBLK, P] so DRAM rows are contiguous nodes
        tpo = ps_aux.tile([NBLK, P], F32, tag="aux", name="tpo")
        nc.tensor.transpose(tpo[:], outn[:], ident_sb[:])
        outT = spool.tile([NBLK, P], F32, tag="outT", name="outT")
        nc.vector.tensor_copy(outT[:], tpo[:])
        nc.sync.dma_start(outd.ap(), outT[:])

    nc.compile()
    return nc


def _prepare(x, edge_index, batch, params, cfg):
    x = np.ascontiguousarray(np.asarray(x, dtype=np.float32))
    src_idx, dst_rel, gid, K = _preprocess(edge_index, batch, cfg)
    nc = _build_nc(cfg, K)

    iota = np.broadcast_to(np.arange(P, dtype=np.float32), (P, P)).copy()
    common = {"x": x, "iota": iota}
    for l, p in enumerate(params):
        do = [P, P, 1][l]
        common[f"w1_{l}"] = np.ascontiguousarray(np.asarray(p["W1"], np.float32))
        common[f"w2_{l}"] = np.ascontiguousarray(np.asarray(p["W2"], np.float32))
        common[f"b1_{l}"] = np.asarray(p["b1"], np.float32).reshape(P, 1)
        common[f"b2_{l}"] = np.asarray(p["b2"], np.float32).reshape(do, 1)
        common[f"gm_{l}"] = np.asarray(p["gamma"], np.float32).reshape(do, 1)
        common[f"bt_{l}"] = np.asarray(p["beta"], np.float32).reshape(do, 1)

    in_maps = []
    for c in range(NCORES):
        m = dict(common)
        m["src_idx"] = src_idx[c]
        m["dst_rel"] = dst_rel[c]
        m["gid"] = gid[c]
        in_maps.append(m)
    return nc, in_maps


def _assemble(results, cfg):
    outs = [results[c]["out"].reshape(-1)[: cfg["NPC"]] for c in range(NCORES)]
    return np.concatenate(outs).reshape(cfg["N"], 1)


def _run(x, edge_index, batch, params, cfg, trace=False):
    nc, in_maps = _prepare(x, edge_index, batch, params, cfg)
    res = bass_utils.run_bass_kernel_spmd(
        nc, in_maps, core_ids=list(range(NCORES)), trace=trace
    )
    return _assemble(res.results, cfg), res


def kernel(x, edge_index, batch, params):
    cfg = _cfg(100000, 64)
    out, _ = _run(x, edge_index, batch, params, cfg)
    return out
